# revision 12
# baseline (speedup 1.0000x reference)
"""BiLSTM-CRF Trainium2 kernel.

Strategy (8 NeuronCores, SPMD -- identical program, per-core data):
  - cores 0-3: forward LSTM for sentence groups 0-3 (16 sentences each)
  - cores 4-7: backward LSTM for sentence groups 0-3 (direction is encoded
    entirely in the per-core input data: reversed gather indices + the
    reverse-direction weights, so the program is identical on all cores)
  - per core: indirect-DMA embedding gather -> PE transpose -> bulk
    x @ W_ih^T + b precompute (PE) -> 256-step LSTM recurrence with
    weights-stationary matmuls, gates kept transposed ([gate, sent] layout)
    so ACT/DVE ops have tiny free dims -> running sum of h_t
  - host: mean-pool projection to 12 tag logits + Viterbi decode (64x12,
    microseconds of numpy)
"""

import sys
import numpy as np
from contextlib import ExitStack

sys.path.insert(0, "/opt/trn_rl_repo")

import concourse.bass as bass
import concourse.bacc as bacc
import concourse.tile as tile
from concourse import mybir
from concourse.bass_utils import run_bass_kernel_spmd
from concourse.masks import make_identity

# problem shapes (fixed by the spec)
B, L, V, E, HD = 64, 256, 50000, 256, 512
HH = HD // 2
T = 12
START, STOP = 10, 11
NEG = -10000.0

NCORES = 8
S = 16          # sentences per core
TL = 8          # timesteps per gather block (TL * S = 128 tokens/block)
SLAB_BLOCKS = 4         # blocks per precompute slab
SLAB_STEPS = TL * SLAB_BLOCKS   # 32 recurrence steps per slab
G4 = 4 * HH     # 1024 gate pre-activations per token
NGC = G4 // 128  # 8 gate chunks of 128
F32 = mybir.dt.float32
DEBUG_MAX_STEPS = None

# gate reorder: torch rows are [i, f, g, o]; we use [i, f, o, g] so that
# sigmoid covers one contiguous 768-wide range and tanh the last 256.
GATE_PERM = np.concatenate([
    np.arange(0, HH), np.arange(HH, 2 * HH),
    np.arange(3 * HH, 4 * HH), np.arange(2 * HH, 3 * HH),
])


def build_program(L_=L, V_=V, num_devices=NCORES):
    """Build the SPMD per-core program. Returns the compiled Bacc object."""
    nblk = L_ // TL
    nslab = nblk // SLAB_BLOCKS

    nc = bacc.Bacc("TRN2", target_bir_lowering=False, debug=False,
                   num_devices=num_devices)

    emb = nc.dram_tensor("emb", [V_, E], F32, kind="ExternalInput").ap()
    idx = nc.dram_tensor("idx", [nblk, 128, 1], mybir.dt.int32,
                         kind="ExternalInput").ap()
    wihT = nc.dram_tensor("wihT", [2, 128, G4], F32, kind="ExternalInput").ap()
    whhT = nc.dram_tensor("whhT", [2, 128, G4], F32, kind="ExternalInput").ap()
    biasr = nc.dram_tensor("biasr", [1, G4], F32, kind="ExternalInput").ap()
    h0T = nc.dram_tensor("h0T", [128, 32], F32, kind="ExternalInput").ap()
    c0T = nc.dram_tensor("c0T", [128, 32], F32, kind="ExternalInput").ap()
    hsumT = nc.dram_tensor("hsumT", [128, 32], F32, kind="ExternalOutput").ap()

    with tile.TileContext(nc) as tc, ExitStack() as ctx:
        const = ctx.enter_context(tc.tile_pool(name="const", bufs=1))
        idxp = ctx.enter_context(tc.tile_pool(name="idxp", bufs=4))
        xsbp = ctx.enter_context(tc.tile_pool(name="xsbp", bufs=4))
        tpp = ctx.enter_context(tc.tile_pool(name="tpp", bufs=2, space="PSUM"))
        xtp = ctx.enter_context(tc.tile_pool(name="xtp", bufs=2))
        xwpp = ctx.enter_context(tc.tile_pool(name="xwpp", bufs=2, space="PSUM"))
        xwp = ctx.enter_context(tc.tile_pool(name="xwp", bufs=2))
        gpp = ctx.enter_context(tc.tile_pool(name="gpp", bufs=2, space="PSUM"))
        actp = ctx.enter_context(tc.tile_pool(name="actp", bufs=3))
        tmpp = ctx.enter_context(tc.tile_pool(name="tmpp", bufs=3))
        hp = ctx.enter_context(tc.tile_pool(name="hp", bufs=3))
        cp = ctx.enter_context(tc.tile_pool(name="cp", bufs=3))
        sump = ctx.enter_context(tc.tile_pool(name="sump", bufs=1))

        # ---- constants ----
        ident = const.tile([128, 128], F32)
        make_identity(nc, ident[:])
        wih_sb = [const.tile([128, G4], F32, tag=f"wih{k}", name=f"wih{k}") for k in range(2)]
        whh_sb = [const.tile([128, G4], F32, tag=f"whh{k}", name=f"whh{k}") for k in range(2)]
        for k in range(2):
            nc.gpsimd.dma_start(out=wih_sb[k][:], in_=wihT[k])
            nc.gpsimd.dma_start(out=whh_sb[k][:], in_=whhT[k])
        bias_sb = const.tile([1, G4], F32)
        nc.gpsimd.dma_start(out=bias_sb[:], in_=biasr[:])
        ones_sb = const.tile([1, 512], F32)
        nc.gpsimd.memset(ones_sb[:], 1.0)
        h0_sb = const.tile([128, 32], F32)
        nc.gpsimd.dma_start(out=h0_sb[:], in_=h0T[:])
        c0_sb = const.tile([128, 32], F32)
        nc.gpsimd.dma_start(out=c0_sb[:], in_=c0T[:])
        hsum = sump.tile([128, 32], F32)
        nc.gpsimd.memset(hsum[:], 0.0)

        # per-engine absorbers: first op on each engine waits on the const
        # producers once so steady-state instructions don't re-accumulate
        # those waits (per-instruction sync-wait slots are scarce).
        junk_ps = tpp.tile([128, 128], F32, tag="tp", padded_shape=[None, 512])
        nc.tensor.transpose(out=junk_ps[:], in_=ident[:], identity=ident[:])
        junk_dve = const.tile([128, 1], F32)
        nc.vector.tensor_copy(out=junk_dve[:], in_=ident[:, 0:1])
        junk_act = const.tile([128, 1], F32)
        nc.scalar.copy(out=junk_act[:], in_=ident[:, 0:1])

        # ---- producers (gather + transpose + xw precompute), emitted in
        # quanta so they interleave with the consuming recurrence ----
        xt_tiles = {}   # slab -> [kc] tiles [128, 512]
        xw_tiles = {}   # slab -> tile [128, SLAB_BLOCKS, TL, NGC, S]

        def emit_block(sl, bl):
            b = sl * SLAB_BLOCKS + bl
            it = idxp.tile([128, 1], mybir.dt.int32, tag="idx")
            nc.gpsimd.dma_start(out=it[:], in_=idx[b])
            xsb = xsbp.tile([128, E], F32, tag="xsb")
            nc.gpsimd.indirect_dma_start(
                out=xsb[:], out_offset=None, in_=emb[:],
                in_offset=bass.IndirectOffsetOnAxis(ap=it[:, :1], axis=0),
            )
            if bl == 0:
                xt_tiles[sl] = [xtp.tile([128, 512], F32, tag=f"xt{k}", name=f"xt{k}_{sl}")
                                for k in range(2)]
            for kc in range(2):
                tp = tpp.tile([128, 128], F32, tag="tp", padded_shape=[None, 512])
                nc.tensor.transpose(out=tp[:], in_=xsb[:, kc * 128:(kc + 1) * 128],
                                    identity=ident[:])
                nc.vector.tensor_copy(
                    out=xt_tiles[sl][kc][:, bl * 128:(bl + 1) * 128], in_=tp[:])

        def emit_precompute_gc(sl, gc):
            if gc == 0:
                xw_tiles[sl] = xwp.tile([128, SLAB_BLOCKS, TL, NGC, S], F32,
                                        tag="xw", name=f"xw_{sl}")
            pw = xwpp.tile([128, SLAB_BLOCKS, TL, S], F32, tag="xwps")
            for kc in range(2):
                nc.tensor.matmul(
                    out=pw[:], lhsT=wih_sb[kc][:, gc * 128:(gc + 1) * 128],
                    rhs=xt_tiles[sl][kc][:], start=(kc == 0), stop=False)
            nc.tensor.matmul(
                out=pw[:], lhsT=bias_sb[0:1, gc * 128:(gc + 1) * 128],
                rhs=ones_sb[0:1, :], start=False, stop=True)
            nc.vector.tensor_copy(out=xw_tiles[sl][:, :, :, gc, :], in_=pw[:])

        # production quanta for one slab: 4 blocks + 8 gc-precomputes
        def production_quanta(sl):
            if sl >= nslab:
                return []
            q = [(lambda s=sl, b=bl: emit_block(s, b)) for bl in range(SLAB_BLOCKS)]
            q += [(lambda s=sl, g=gc: emit_precompute_gc(s, g)) for gc in range(NGC)]
            return q

        # ---- recurrence ----
        h_cur, c_cur = h0_sb, c0_sb

        def emit_step(sl, sis):
            nonlocal h_cur, c_cur
            bl, tl = sis // TL, sis % TL
            # two PSUM tiles (separate banks): sigmoid gates (i,f,o) and g-tilde,
            # so each accumulation group closes before its reader runs.
            g1 = gpp.tile([128, 6, S], F32, tag="g1", padded_shape=[None, 32, None])
            g2 = gpp.tile([128, 2, S], F32, tag="g2", padded_shape=[None, 32, None])
            nc.tensor.matmul(out=g1[:], lhsT=ident[:],
                             rhs=xw_tiles[sl][:, bl, tl, 0:6, :],
                             start=True, stop=False, skip_group_check=True)
            nc.tensor.matmul(out=g2[:], lhsT=ident[:],
                             rhs=xw_tiles[sl][:, bl, tl, 6:8, :],
                             start=True, stop=False, skip_group_check=True)
            for hc in range(2):
                for gc in (6, 7, 0, 1, 2, 3, 4, 5):
                    dst = g1[:, gc, :] if gc < 6 else g2[:, gc - 6, :]
                    nc.tensor.matmul(
                        out=dst,
                        lhsT=whh_sb[hc][:, gc * 128:(gc + 1) * 128],
                        rhs=h_cur[:, hc * 16:(hc + 1) * 16],
                        start=False, stop=(hc == 1),
                        skip_group_check=True)
            gv = g1[:].rearrange("p gc s -> p (gc s)")
            sifo = actp.tile([128, 96], F32, tag="sifo")
            nc.scalar.activation(sifo[:], gv[:],
                                 mybir.ActivationFunctionType.Sigmoid)
            gt = actp.tile([128, 32], F32, tag="gt")
            nc.scalar.activation(gt[:], g2[:].rearrange("p gc s -> p (gc s)"),
                                 mybir.ActivationFunctionType.Tanh)
            t1 = tmpp.tile([128, 32], F32, tag="t1")
            nc.vector.tensor_tensor(out=t1[:], in0=sifo[:, 32:64], in1=c_cur[:],
                                    op=mybir.AluOpType.mult)
            t2 = tmpp.tile([128, 32], F32, tag="t2")
            nc.vector.tensor_tensor(out=t2[:], in0=sifo[:, 0:32], in1=gt[:],
                                    op=mybir.AluOpType.mult)
            c_new = cp.tile([128, 32], F32, tag="c")
            nc.vector.tensor_tensor(out=c_new[:], in0=t1[:], in1=t2[:],
                                    op=mybir.AluOpType.add)
            tc_t = tmpp.tile([128, 32], F32, tag="tc")
            nc.scalar.activation(tc_t[:], c_new[:],
                                 mybir.ActivationFunctionType.Tanh)
            h_new = hp.tile([128, 32], F32, tag="h")
            nc.vector.tensor_tensor(out=h_new[:], in0=sifo[:, 64:96], in1=tc_t[:],
                                    op=mybir.AluOpType.mult)
            nc.vector.tensor_tensor(out=hsum[:], in0=hsum[:], in1=h_new[:],
                                    op=mybir.AluOpType.add)
            h_cur, c_cur = h_new, c_new

        # slab 0 produced up front (pipeline fill)
        nstep_emitted = 0
        for q in production_quanta(0):
            q()
        for sl in range(nslab):
            quanta = production_quanta(sl + 1)
            nq = len(quanta)
            for sis in range(SLAB_STEPS):
                if DEBUG_MAX_STEPS is not None and nstep_emitted >= DEBUG_MAX_STEPS:
                    break
                # spread next-slab production across this slab's steps
                while nq and len(quanta) > (SLAB_STEPS - 1 - sis) * nq // SLAB_STEPS:
                    quanta.pop(0)()
                emit_step(sl, sis)
                nstep_emitted += 1

        nc.gpsimd.dma_start(out=hsumT[:], in_=hsum[:])

    nc.compile()
    return nc


def host_inputs(sentences, emb, w_ih, w_hh, b_ih, b_hh,
                w_ih_r, w_hh_r, b_ih_r, b_hh_r, h0, c0, L_=L):
    """Build the 8 per-core input dicts (all layout prep on host)."""
    nblk = L_ // TL
    sentences = np.asarray(sentences)
    emb = np.ascontiguousarray(np.asarray(emb, dtype=np.float32))

    def wprep(w):
        return np.ascontiguousarray(
            np.asarray(w, dtype=np.float32)[GATE_PERM].T.reshape(2, 128, G4))

    def bprep(bi, bh):
        return np.ascontiguousarray(
            (np.asarray(bi, dtype=np.float32)
             + np.asarray(bh, dtype=np.float32))[GATE_PERM].reshape(1, G4))

    wih_f, whh_f, b_f = wprep(w_ih), wprep(w_hh), bprep(b_ih, b_hh)
    wih_b, whh_b, b_b = wprep(w_ih_r), wprep(w_hh_r), bprep(b_ih_r, b_hh_r)

    h0 = np.asarray(h0, dtype=np.float32)
    c0 = np.asarray(c0, dtype=np.float32)

    in_maps = []
    for core in range(NCORES):
        d = core // 4            # 0 = fwd, 1 = bwd
        grp = core % 4
        sent = sentences[grp * S:(grp + 1) * S]          # [S, L]
        # token position for (block b, tl, s): t = b*TL + tl (fwd) or
        # L-1 - (b*TL + tl) (bwd)
        tpos = (np.arange(nblk)[:, None] * TL + np.arange(TL)[None, :]).reshape(-1)
        if d == 1:
            tpos = (L_ - 1) - tpos
        ids = sent[:, tpos].T.reshape(nblk, TL, S)        # [nblk, TL, S]
        idx = np.ascontiguousarray(
            ids.reshape(nblk, 128, 1).astype(np.int32))
        # state layout [128, kc*16 + s] = state[s, d, kc*128 + p]
        st = lambda a: np.ascontiguousarray(
            a[grp * S:(grp + 1) * S, d, :].reshape(S, 2, 128)
             .transpose(2, 1, 0).reshape(128, 32).astype(np.float32))
        in_maps.append(dict(
            emb=emb, idx=idx,
            wihT=wih_f if d == 0 else wih_b,
            whhT=whh_f if d == 0 else whh_b,
            biasr=b_f if d == 0 else b_b,
            h0T=st(h0), c0T=st(c0),
        ))
    return in_maps


def hsum_from_outputs(results):
    """Reassemble [B, HD] sum-over-t of [hf, hb] from the 8 core outputs."""
    hs = np.zeros((B, HD), dtype=np.float32)
    for core in range(NCORES):
        d, grp = core // 4, core % 4
        o = results[core]["hsumT"]                      # [128, 32]
        # o[p, kc*16+s] = sum_t h[s, kc*128+p]
        part = o.reshape(128, 2, S).transpose(2, 1, 0).reshape(S, 256)
        hs[grp * S:(grp + 1) * S, d * HH:(d + 1) * HH] = part
    return hs


def viterbi_host(feats, transitions):
    feats = np.asarray(feats, dtype=np.float32)
    transitions = np.asarray(transitions, dtype=np.float32)
    fv = np.full((T,), NEG, dtype=np.float32)
    fv[START] = np.float32(0.0)
    bps = np.zeros((feats.shape[0], T), dtype=np.int32)
    for i in range(feats.shape[0]):
        ntv = fv[None, :] + transitions          # [T, T]
        bps[i] = np.argmax(ntv, axis=1)
        fv = ntv[np.arange(T), bps[i]] + feats[i]
    terminal = fv + transitions[STOP]
    best = int(np.argmax(terminal))
    path_score = terminal[best]
    path = [best]
    for i in range(feats.shape[0] - 1, -1, -1):
        path.append(int(bps[i][path[-1]]))
    assert path[-1] == START
    tag_seq = np.array(path[-2::-1], dtype=np.int32)
    return np.float32(path_score), tag_seq


_PROG = None


def kernel(sentences, sentence_lens, emb, w_ih, w_hh, b_ih, b_hh,
           w_ih_r, w_hh_r, b_ih_r, b_hh_r, w_tag, b_tag,
           transitions, h0, c0, _trace=False):
    global _PROG
    if _PROG is None:
        _PROG = build_program()
    in_maps = host_inputs(sentences, emb, w_ih, w_hh, b_ih, b_hh,
                          w_ih_r, w_hh_r, b_ih_r, b_hh_r, h0, c0)
    res = run_bass_kernel_spmd(_PROG, in_maps, core_ids=list(range(NCORES)),
                               trace=_trace)
    hs = hsum_from_outputs(res.results)
    w_tag = np.asarray(w_tag, dtype=np.float32)
    b_tag = np.asarray(b_tag, dtype=np.float32)
    feats = (hs / np.float32(L)) @ w_tag.T + b_tag       # [B, T]
    out = viterbi_host(feats, np.asarray(transitions, dtype=np.float32))
    if _trace:
        return out, res
    return out


def make_runner(prog, in_maps):
    """Build a reusable jitted runner mirroring bass2jax.run_bass_via_pjrt's
    multi-core path, with inputs staged on device once. Returns (run, n_outs).
    Each run() re-supplies fresh donated zero outputs and blocks."""
    import jax
    from jax.sharding import Mesh, PartitionSpec
    from jax.experimental.shard_map import shard_map
    from concourse import bass2jax as b2j
    from concourse import mybir as _mb

    b2j.install_neuronx_cc_hook()
    nc = prog
    n_cores = len(in_maps)
    partition_name = nc.partition_id_tensor.name if nc.partition_id_tensor else None
    in_names, out_names, out_avals, zero_outs = [], [], [], []
    for alloc in nc.m.functions[0].allocations:
        if not isinstance(alloc, _mb.MemoryLocationSet):
            continue
        name = alloc.memorylocations[0].name
        if alloc.kind == "ExternalInput":
            if name != partition_name:
                in_names.append(name)
        elif alloc.kind == "ExternalOutput":
            shape = tuple(alloc.tensor_shape)
            dtype = _mb.dt.np(alloc.dtype)
            out_avals.append(jax.core.ShapedArray(shape, dtype))
            zero_outs.append(np.zeros(shape, dtype))
            out_names.append(name)
    n_params = len(in_names)
    n_outs = len(out_avals)
    all_in_names = list(in_names) + list(out_names)
    if partition_name is not None:
        all_in_names.append(partition_name)
    donate = tuple(range(n_params, n_params + n_outs))

    def _body(*args):
        operands = list(args)
        if partition_name is not None:
            operands.append(b2j.partition_id_tensor())
        outs = b2j._bass_exec_p.bind(
            *operands, out_avals=tuple(out_avals), in_names=tuple(all_in_names),
            out_names=tuple(out_names), lowering_input_output_aliases=(),
            sim_require_finite=True, sim_require_nnan=True, nc=nc)
        return tuple(outs)

    devices = jax.devices()[:n_cores]
    mesh = Mesh(np.asarray(devices), ("core",))
    in_specs = (PartitionSpec("core"),) * (n_params + n_outs)
    out_specs = (PartitionSpec("core"),) * n_outs
    sharded = jax.jit(
        shard_map(_body, mesh=mesh, in_specs=in_specs, out_specs=out_specs,
                  check_rep=False),
        donate_argnums=donate, keep_unused=True)
    from jax.sharding import NamedSharding
    shd = NamedSharding(mesh, PartitionSpec("core"))
    concat_in = [
        jax.device_put(np.concatenate([np.asarray(in_maps[c][nm]) for c in range(n_cores)], axis=0), shd)
        for nm in in_names]
    concat_zero_shapes = [( n_cores * z.shape[0], *z.shape[1:]) for z in zero_outs]
    zdtypes = [z.dtype for z in zero_outs]

    def run():
        zeros = [jax.device_put(np.zeros(s_, d_), shd)
                 for s_, d_ in zip(concat_zero_shapes, zdtypes)]
        outs = sharded(*concat_in, *zeros)
        jax.block_until_ready(outs)
        return outs

    return run, out_names, out_avals
